# revision 1
# baseline (speedup 1.0000x reference)
"""Trainium2 Bass kernel for nn_AttentionDecoder (ragged attention decoder scores).

Reference computation:
    padded = action_embed[gather_idx] * valid_mask[..., None]   # [B, M, D]
    q = state_embed @ wq                                        # [B, D]
    k = padded @ wk                                             # [B, M, D]
    scores = einsum("bd,bmd->bm", q, k)                         # [B, M]
    out = scores.reshape(-1)[rev_idx][:, None]                  # [total, 1]

Algebra: scores[b,m] = padded[b,m] @ (wq @ wk^T)^T @ state[b]^T. With
z = state_embed @ (wq @ wk^T), the per-node output is
    out[i] = action_embed[i] . z[graph(i)]
for the deterministic ragged layout produced by setup_inputs() (gather_idx is
a contiguous ragged gather, rev_idx the inverse permutation, valid_mask only
kills padded slots that never reach the output).

Sharding: data-parallel over graphs. Core c gets graphs [2048c, 2048(c+1))
and the matching contiguous node range [25600c, 25600(c+1)) (the count
pattern 5 + b%16 sums to 200 per 16 graphs, so every core gets exactly
25600 nodes). wq/wk replicated.

Per-core device program (fp32 data; the 0/1-weighted expansion and
reduction matmuls run as f32r, 1 column/cycle vs fp32's 4, costing only a
~2^-11 mantissa truncation of the streamed operand):
    W  = wq @ wk^T                (PE, via host-transposed wqT/wkT)
    z  = S @ W                    (PE, via host-transposed St)
    for each 512-node block i:
        zx   = z-rows expanded to nodes    (PE: z_tile^T @ Sel, static 0/1 Sel)
        prod = At_block * zx               (DVE elementwise, zx from PSUM)
        score[i, :] += ones-column matmul  (PE: reduces over d=128 partitions,
                                            lands block i's 512 scores in
                                            PSUM partition i)
    copy PSUM scores -> SBUF -> DRAM out [50, 512]

The kernel is DMA-bound: ~14.5 MB/core of streamed input against ~360 GB/s
per-core HBM bandwidth sets a ~42 us floor; the cost model puts this program
at ~47.6 us with PE/DVE/ACT overlapped underneath the At stream. Cycling
buffers are rotated manually (no pool slot recycling) and excess semaphore
waits are split onto EventSemaphore ops because this toolchain's walrus
accepts at most one sync wait per regular instruction.
"""

import numpy as np

B = 16384
M = 20
D = 128
NCORES = 8
GPC = B // NCORES            # graphs per core = 2048
COUNTS = 5 + (np.arange(B) % 16)
NPC = 25600                  # nodes per core (sum of counts over 2048 graphs)
TOTAL = int(COUNTS.sum())    # 204800
BLK = 512                    # nodes per block
NBLK = NPC // BLK            # 50
SEL_PERIOD = 1600            # node period of the (graph id mod 128) pattern
SEL_COLS = SEL_PERIOD + BLK  # padded so any 512-window is a contiguous slice


def _graph_of_node_local():
    """graph id (local to a core) for each of the 25600 local nodes."""
    counts = 5 + (np.arange(GPC) % 16)
    return np.repeat(np.arange(GPC), counts)


_GL = _graph_of_node_local()


def _sel_matrix():
    """[128, SEL_COLS] fp32; col s has a 1 at row (graph_of_node(s) % 128)."""
    sel = np.zeros((128, SEL_PERIOD), np.float32)
    sel[_GL[:SEL_PERIOD] % 128, np.arange(SEL_PERIOD)] = 1.0
    return np.concatenate([sel, sel[:, :BLK]], axis=1).copy()


def _ones_pad():
    """[128, 256] fp32, column 128 is all-ones: window [128-i, 256-i) is the
    ones-column-at-i stationary operand for the reduction matmul of block i."""
    o = np.zeros((128, 256), np.float32)
    o[:, 128] = 1.0
    return o


def _block_pieces():
    """Static per-block expansion plan.

    For block i, returns a list of (z_tile, sel_col, out_col, width):
    matmul(out=zx[:, out_col:out_col+width],
           lhsT=z_sb[:, 128*z_tile : 128*z_tile+128],
           rhs=sel_sb[:, sel_col : sel_col+width])
    A block splits into two pieces when its 512 nodes straddle a 128-graph
    (= 1600-node) boundary.
    """
    plans = []
    for i in range(NBLK):
        lo = BLK * i
        base = lo % SEL_PERIOD
        k0 = int(_GL[lo]) // 128
        # node index where graph id reaches the next multiple of 128
        nxt = (int(_GL[lo]) // 128 + 1) * 128
        pieces = []
        if nxt <= int(_GL[lo + BLK - 1]):
            split = int(np.searchsorted(_GL[lo:lo + BLK], nxt))
            pieces.append((k0, base, 0, split))
            pieces.append((k0 + 1, base + split, split, BLK - split))
        else:
            pieces.append((k0, base, 0, BLK))
        plans.append(pieces)
    return plans


_PIECES = _block_pieces()

# PE dtype for the expansion (z-row broadcast) and reduction (sum over d)
# matmuls. "f32r" streams 1 column/cycle vs fp32's 4 (fp32 runs as two
# half-rate passes); both multiply by exact 0/1 constants, so the only
# precision cost is mantissa truncation of the data operand (~2^-11).
EXP_F32R = True
RED_F32R = True

# Every Nth block's product runs on GpSimd (0 = all products on DVE); the
# EventSemaphore wait-split pass absorbs the extra cross-engine waits this
# creates.
GPSIMD_EVERY = 3

_PROGRAM = None


def _build_program(split_waits=True, reps=1, loop_reps=1):
    import concourse.bass as bass
    import concourse.tile as tile
    from concourse import mybir
    from contextlib import ExitStack

    f32 = mybir.dt.float32
    f32r = mybir.dt.float32r
    nc = bass.Bass("TRN2", target_bir_lowering=False, debug=False,
                   use_seq_codegen=True)

    at_d = nc.dram_tensor("at", [128, NPC], f32, kind="ExternalInput").ap()
    assert EXP_F32R == RED_F32R  # sel and ones share one packed input
    sel_dt = f32r if EXP_F32R else f32
    red_dt = sel_dt
    # One DMA per dtype group keeps every PE matmul at <=1 semaphore wait
    # (walrus allows a single sync wait on the lowered LDWEIGHTS+MATMULT).
    # cstf = [wqt | wkt | st], cstr = [sel | ones].
    cstf_d = nc.dram_tensor("cstf", [128, 256 + GPC], f32,
                            kind="ExternalInput").ap()
    bf16 = mybir.dt.bfloat16
    # sel ships as bf16 (0/1 exact) and is cast to f32r on-device; ones is
    # memset on-device - both halve/remove startup DMA bytes.
    cstr_d = nc.dram_tensor("cstr", [128, SEL_COLS], bf16,
                            kind="ExternalInput").ap()
    out_d = nc.dram_tensor("out", [NBLK, BLK], f32, kind="ExternalOutput").ap()

    ZT = GPC // 128  # 16 z tiles

    with tile.TileContext(nc) as tc, ExitStack() as ctx:
        # All cycling buffers are allocated once and rotated manually.  Pool
        # slot recycling creates cross-engine release waits, and this walrus
        # build allows only ONE sync wait per instruction; with fixed tiles
        # the steady-state waits are exactly: PE waits DVE (zx reuse), DVE
        # waits PE (zx produced), and everything older is covered by the
        # monotonicity of those two semaphores.
        consts = ctx.enter_context(tc.tile_pool(name="consts", bufs=1))
        psum = ctx.enter_context(tc.tile_pool(name="psum", bufs=1, space="PSUM"))

        # At chunk schedule: a small first chunk lets compute start early and
        # small last chunks shrink the DMA-idle tail; one dedicated tile per
        # chunk (no reuse within a pass).
        CHUNKS = [4, 12, 12, 12, 6, 2, 2]
        CSTART = [0]
        for n in CHUNKS:
            CSTART.append(CSTART[-1] + n)
        assert CSTART[-1] == NBLK
        NZX = 5     # zx PSUM tiles (1 bank each); 5 divides NBLK
        NPROD = 10  # prod SBUF tiles; 10 divides NBLK

        cstf_sb = consts.tile([128, 256 + GPC], f32, tag="cstf")
        nc.scalar.dma_start(out=cstf_sb[:], in_=cstf_d[:])
        cstr_sb = consts.tile([128, SEL_COLS], bf16, tag="cstr")
        nc.scalar.dma_start(out=cstr_sb[:], in_=cstr_d[:])
        wqt_sb = cstf_sb[:, 0:128]
        wkt_sb = cstf_sb[:, 128:256]
        st_sb = cstf_sb[:, 256:256 + GPC]
        selr_sb = consts.tile([128, SEL_COLS], sel_dt, tag="selr")
        nc.scalar.copy(selr_sb[:], cstr_sb[:])
        sel_sb = selr_sb[:]
        ones_st = consts.tile([128, 256], f32, tag="ones_s")
        nc.gpsimd.memset(ones_st[:], 0.0)
        nc.gpsimd.memset(ones_st[:, 128:129], 1.0)
        ones_sb = consts.tile([128, 256], red_dt, tag="ones")
        nc.scalar.copy(ones_sb[:], ones_st[:])
        w_sb = consts.tile([128, 128], f32, tag="w")
        z_sb = consts.tile([128, GPC], sel_dt, tag="z")
        touch_sb = consts.tile([1, 16], f32, tag="touch")

        at_tiles = [consts.tile([128, n * BLK], f32, tag=f"at{j}", name=f"at{j}")
                    for j, n in enumerate(CHUNKS)]
        prod_tiles = [consts.tile([128, BLK], red_dt, tag=f"prod{j}", name=f"prod{j}")
                      for j in range(NPROD)]
        zxs_tiles = [consts.tile([128, BLK], f32, tag=f"zxs{j}", name=f"zxs{j}")
                     for j in range(3)]
        # PSUM: zx 5 banks + score 1 = 6 of 8; the z phase borrows the zx
        # tiles before the main loop starts.
        zx_tiles = [psum.tile([128, BLK], f32, tag=f"zx{j}", name=f"zx{j}") for j in range(NZX)]
        # Two score banks: blocks 0-24 accumulate in sc0 which is copied and
        # stored while the At stream is still running; sc1 holds the rest.
        sc0_ps = psum.tile([128, BLK], f32, tag="score0")
        sc1_ps = psum.tile([128, BLK], f32, tag="score1")
        HALF = NBLK // 2

        # W = wq @ wk^T lands in zx[4] and is copied out before the z matmuls
        # need it (their read of w_sb orders them).
        nc.tensor.matmul(zx_tiles[4][:, 0:128], lhsT=wqt_sb[:], rhs=wkt_sb[:],
                         start=True, stop=True)
        # Fence: advance PE's clock past the ones memsets (Pool) with a
        # throwaway matmul so the first reduction matmul needs no Pool wait.
        nc.tensor.matmul(zx_tiles[4][0:1, 128:129],
                         lhsT=ones_sb[:, 0:1].bitcast(f32),
                         rhs=ones_sb[:, 0:1].bitcast(f32), start=True, stop=True)
        nc.scalar.copy(w_sb[:], zx_tiles[4][:, 0:128])

        # z = S @ W : out[g, d] = sum_a St[a, g] * W[a, d]; 16 tiles of 128
        # graphs across zx[0..3], one ACT copy per PSUM tile.
        for q in range(ZT // 4):
            for m in range(4):
                k = 4 * q + m
                nc.tensor.matmul(zx_tiles[q][:, 128 * m:128 * m + 128],
                                 lhsT=st_sb[:, 128 * k:128 * k + 128],
                                 rhs=w_sb[:], start=True, stop=True)
            nc.scalar.copy(z_sb[:, 512 * q:512 * q + 512], zx_tiles[q][:])

        # Second fence: reading the last z tile advances PE's clock past the
        # ACT z copies, so expansion matmuls never wait on ACT mid-loop.
        nc.tensor.matmul(zx_tiles[4][0:1, 129:130],
                         lhsT=z_sb[:, GPC - 1:GPC].bitcast(f32),
                         rhs=z_sb[:, GPC - 1:GPC].bitcast(f32), start=True, stop=True)

        from contextlib import nullcontext
        at_chunk = None
        coff = 0
        loop_cm = tc.For_i(0, loop_reps, 1) if loop_reps > 1 else nullcontext()
        with loop_cm:
            for r in range(reps):
                cidx = -1
                for i in range(NBLK):
                    if i in CSTART[:-1]:
                        cidx += 1
                        at_chunk = at_tiles[cidx]
                        coff = i
                        lo = BLK * i
                        hi = BLK * CSTART[cidx + 1]
                        # HWDGE on the SP queue: each chunk writes its own
                        # dedicated tile (no WAW between chunks), and the DVE
                        # touch below absorbs the single completion wait for
                        # this chunk's product TTs.
                        nc.sync.dma_start(out=at_chunk[:, :hi - lo],
                                          in_=at_d[:, lo:hi])
                        nc.vector.tensor_copy(touch_sb[0:1, 0:1],
                                              at_chunk[0:1, 0:1])
                    at_t = at_chunk[:, BLK * (i - coff):BLK * (i - coff) + BLK]

                    zx = zx_tiles[i % NZX]
                    for (k, scol, ocol, w) in _PIECES[i]:
                        nc.tensor.matmul(zx[:, ocol:ocol + w],
                                         lhsT=z_sb[:, 128 * k:128 * k + 128],
                                         rhs=sel_sb[:, scol:scol + w],
                                         start=True, stop=True)

                    prod = prod_tiles[i % NPROD]
                    if GPSIMD_EVERY and i % GPSIMD_EVERY == GPSIMD_EVERY - 1:
                        # GpSimd cannot read PSUM; ACT stages zx into SBUF.
                        zxs = zxs_tiles[(i // GPSIMD_EVERY) % len(zxs_tiles)]
                        nc.scalar.copy(zxs[:], zx[:])
                        nc.gpsimd.tensor_mul(prod[:], at_t[:], zxs[:])
                    else:
                        nc.vector.tensor_mul(prod[:], at_t[:], zx[:])

                    sc_ps = sc0_ps if i < HALF else sc1_ps
                    ii = i % HALF
                    nc.tensor.matmul(sc_ps[:],
                                     lhsT=ones_sb[:, 128 - ii:256 - ii],
                                     rhs=prod[:],
                                     start=(ii == 0),
                                     stop=(ii == HALF - 1))
                    if i == HALF - 1:
                        out0_sb = consts.tile([HALF, BLK], f32, tag="out0",
                                              name=f"out0_{r}")
                        nc.scalar.copy(out0_sb[:], sc0_ps[0:HALF, :])
                        nc.sync.dma_start(out=out_d[0:HALF, :], in_=out0_sb[:])

        out_sb = consts.tile([NBLK - NBLK // 2, BLK], f32, tag="out")
        nc.scalar.copy(out_sb[:], sc1_ps[0:NBLK - NBLK // 2, :])
        nc.sync.dma_start(out=out_d[NBLK // 2:NBLK, :], in_=out_sb[:])

    if split_waits:
        _split_multi_waits(nc)
    return nc


def _split_multi_waits(nc):
    """Walrus in this toolchain accepts at most one sync wait on a regular
    instruction (and two on an EventSemaphore). Tile's sem assignment can
    attach several, so strip the excess onto same-engine EventSemaphore
    instructions placed immediately before the owner - same-engine program
    order makes that equivalent."""
    from concourse import mybir
    for fn in nc.m.functions:
        for bb in fn.blocks:
            new = []
            for inst in bb.instructions:
                si = inst.sync_info
                if (si is not None and len(si.on_wait) > 1
                        and not isinstance(inst, mybir.InstEventSemaphore)):
                    waits = list(si.on_wait)
                    keep, rest = waits[-1:], waits[:-1]
                    k = 0
                    while rest:
                        chunk, rest = rest[:2], rest[2:]
                        new.append(mybir.InstEventSemaphore(
                            name=f"{inst.name}-w{k}",
                            engine=inst.engine,
                            sync_info=mybir.SyncInfo(on_wait=chunk,
                                                     on_update=[])))
                        k += 1
                    inst.sync_info = mybir.SyncInfo(
                        on_wait=keep, on_update=list(si.on_update))
                new.append(inst)
            bb.instructions[:] = new


def _get_program():
    global _PROGRAM
    if _PROGRAM is None:
        _PROGRAM = _build_program()
    return _PROGRAM


def _structured(gather_idx, valid_mask, rev_idx):
    """True iff the index tensors match the deterministic ragged layout."""
    counts = COUNTS
    off = np.concatenate([[0], np.cumsum(counts)[:-1]])
    slots = np.arange(M)[None, :]
    valid = (slots < counts[:, None])
    gidx = off[:, None] + np.minimum(slots, counts[:, None] - 1)
    within = np.arange(TOTAL) - np.repeat(off, counts)
    rev = np.repeat(np.arange(B), counts) * M + within
    return (np.array_equal(np.asarray(gather_idx), gidx)
            and np.array_equal(np.asarray(valid_mask), valid.astype(np.float32))
            and np.array_equal(np.asarray(rev_idx), rev))


def _reference_fallback(state_embed, action_embed, wq, wk, gather_idx,
                        valid_mask, rev_idx):
    padded = action_embed[gather_idx] * valid_mask[..., None]
    q = state_embed @ wq
    k = padded @ wk
    scores = np.einsum("bd,bmd->bm", q, k)
    return scores.reshape(-1)[rev_idx][:, None].astype(np.float32)


def _make_in_maps(ins):
    state_embed = np.ascontiguousarray(np.asarray(ins["state_embed"], np.float32))
    action_embed = np.ascontiguousarray(np.asarray(ins["action_embed"], np.float32))
    wqt = np.asarray(ins["wq"], np.float32).T
    wkt = np.asarray(ins["wk"], np.float32).T
    cstr = _sel_matrix().astype(np.float32).astype(
        __import__("ml_dtypes").bfloat16)                        # [128, 2112]
    in_maps = []
    for c in range(NCORES):
        st_c = state_embed[GPC * c:GPC * (c + 1)].T             # [128, 2048]
        at_c = np.ascontiguousarray(
            action_embed[NPC * c:NPC * (c + 1)].T)              # [128, 25600]
        cstf = np.ascontiguousarray(
            np.concatenate([wqt, wkt, st_c], axis=1))           # [128, 2304]
        in_maps.append({"at": at_c, "cstf": cstf, "cstr": cstr})
    return in_maps


def kernel(state_embed, action_embed, wq, wk, gather_idx, valid_mask, rev_idx):
    if not _structured(gather_idx, valid_mask, rev_idx):
        # Inputs deviate from the deterministic ragged layout this kernel is
        # specialized for; fall back to a host computation to stay correct.
        return _reference_fallback(
            np.asarray(state_embed, np.float32),
            np.asarray(action_embed, np.float32),
            np.asarray(wq, np.float32), np.asarray(wk, np.float32),
            np.asarray(gather_idx), np.asarray(valid_mask),
            np.asarray(rev_idx))

    from concourse.bass_utils import run_bass_kernel_spmd

    nc = _get_program()
    in_maps = _make_in_maps({
        "state_embed": state_embed, "action_embed": action_embed,
        "wq": wq, "wk": wk,
    })
    results = run_bass_kernel_spmd(nc, in_maps, list(range(NCORES))).results
    out = np.concatenate([results[c]["out"].reshape(-1) for c in range(NCORES)])
    return out[:, None]



# revision 11
# speedup vs baseline: 4.1139x; 4.1139x over previous
"""Trainium2 Bass kernel for nn_AttentionDecoder (ragged attention decoder scores).

Reference computation:
    padded = action_embed[gather_idx] * valid_mask[..., None]   # [B, M, D]
    q = state_embed @ wq                                        # [B, D]
    k = padded @ wk                                             # [B, M, D]
    scores = einsum("bd,bmd->bm", q, k)                         # [B, M]
    out = scores.reshape(-1)[rev_idx][:, None]                  # [total, 1]

Algebra: with z = state_embed @ (wq @ wk^T), the per-node output is
    out[i] = action_embed[i] . z[graph(i)]
for the deterministic ragged layout produced by setup_inputs() (gather_idx is
a contiguous ragged gather, rev_idx the inverse permutation, valid_mask only
kills padded slots that never reach the output).

Sharding: data-parallel over graphs. Core c gets graphs [2048c, 2048(c+1))
and the matching contiguous node range [25600c, 25600(c+1)) (the count
pattern 5 + b%16 sums to 200 per 16 graphs, so every core gets exactly
25600 nodes). wq/wk replicated.

Per-core device program:
    The At stream (node embeddings, transposed to [128 d, 25600 nodes] and
    cast to bf16 on host) dominates the data volume.  It is split across all
    three DMA-capable queues (SP, Activation, Pool/SWDGE), which stream
    concurrently; Pool first brings in wq/wk (f32) and S^T (bf16).

    PE computes W = wq @ wk^T, then z^T = W^T S^T (bf16 stationary W,
    streaming S^T), staged to SBUF as bf16 by DVE.  Scores are then produced
    by one small matmul per graph: stationary = the graph's At columns
    [128 d, c_g], moving = its z column [128 d, 1], output = [c_g, 1].
    Matmul outputs may only start at PSUM partition 0/32/64, so graph g
    lands at partition base 32*(g%3), column g//3 — 683 columns over two
    PSUM banks, zero-initialised once per bank (start=True) with every
    graph accumulating start=False into its disjoint slot.  A [128, 683]
    f32 copy stages the scores and three parallel stores (one per queue)
    emit them; the host gathers the 3-band layout back to node order.

    bf16 inputs keep every matmul at 1 column/cycle and halve the HBM
    traffic; the quantisation error (~2^-9 relative per operand) leaves an
    order of magnitude of margin against the 2e-2 gate.
"""

import numpy as np

B = 16384
M = 20
D = 128
NCORES = 8
GPC = B // NCORES            # graphs per core = 2048
COUNTS = 5 + (np.arange(B) % 16)
NPC = 25600                  # nodes per core (sum of counts over 2048 graphs)
TOTAL = int(COUNTS.sum())    # 204800
NCOL = (GPC + 2) // 3        # 683 PSUM score columns (3 graphs per column)

# At stream chunking: units of 1600 nodes (aligned to whole graphs; 16 units
# per core) distributed over the three DMA queues.  Pool carries the wq/wk +
# S^T preamble, so it gets fewer At units.
UNIT = 1600
SP_UNITS = list(range(0, 6))      # nodes [0, 9600)
ACT_UNITS = list(range(6, 12))    # nodes [9600, 19200)
POOL_UNITS = list(range(12, 16))  # nodes [19200, 25600)


def _graph_layout():
    """Per-core graph layout: (offsets[2049], counts[2048]) in local nodes."""
    counts = 5 + (np.arange(GPC) % 16)
    off = np.concatenate([[0], np.cumsum(counts)])
    return off, counts


_OFF, _CNT = _graph_layout()


def _graph_pieces():
    """Static per-graph score-matmul plan.

    Returns a list of (node0, width, psum_col, psum_row, graph) in graph
    order: matmul(out=sc[psum_row:psum_row+width, psum_col:psum_col+1],
    lhsT=at[:, node0:node0+width], rhs=z[:, graph:graph+1]).  Matmul outputs
    may only start at PSUM partition 0/32/64, so graph g goes to column
    g//3 at partition base 32*(g%3) (counts <= 20 < 32 always fit).
    """
    return [(int(_OFF[g]), int(_CNT[g]), g // 3, 32 * (g % 3), g)
            for g in range(GPC)]


_PIECES = _graph_pieces()


def _out_gather_index():
    """(rows, cols) gathering the [128, NCOL] device output to node order."""
    g = np.repeat(np.arange(GPC), _CNT)
    j = np.arange(NPC) - np.repeat(_OFF[:-1], _CNT)
    return 32 * (g % 3) + j, g // 3


_OROWS, _OCOLS = _out_gather_index()

_PROGRAM = None


def _build_program(split_waits=True):
    import concourse.bass as bass
    import concourse.tile as tile
    from concourse import mybir
    from contextlib import ExitStack

    f32 = mybir.dt.float32
    bf16 = mybir.dt.bfloat16
    nc = bass.Bass("TRN2", target_bir_lowering=False, debug=False,
                   use_seq_codegen=True)

    at_d = nc.dram_tensor("at", [128, NPC], bf16, kind="ExternalInput").ap()
    st_d = nc.dram_tensor("st", [128, GPC], bf16, kind="ExternalInput").ap()
    wqwk_d = nc.dram_tensor("wqwk", [128, 256], f32, kind="ExternalInput").ap()
    out_d = nc.dram_tensor("out", [128, NCOL], f32, kind="ExternalOutput").ap()

    with tile.TileContext(nc) as tc, ExitStack() as ctx:
        consts = ctx.enter_context(tc.tile_pool(name="consts", bufs=1))
        psum = ctx.enter_context(tc.tile_pool(name="psum", bufs=1, space="PSUM"))

        at_sb = consts.tile([128, NPC], bf16, tag="at")
        st_sb = consts.tile([128, GPC], bf16, tag="st")
        wqwk_sb = consts.tile([128, 256], f32, tag="wqwk")
        w_sb = consts.tile([128, 128], bf16, tag="w")
        z_sb = consts.tile([128, GPC], bf16, tag="z")
        o_sb = consts.tile([128, NCOL], f32, tag="o")

        w_ps = psum.tile([128, 128], f32, tag="w_ps")
        zt_ps = psum.tile([128, GPC], f32, tag="zt_ps")
        sc_ps = psum.tile([128, 1024], f32, tag="sc_ps")

        # Preamble on the Pool queue; At units stream on all three queues.
        nc.gpsimd.dma_start(out=wqwk_sb[:], in_=wqwk_d[:])
        nc.gpsimd.dma_start(out=st_sb[:], in_=st_d[:])
        for units, q in ((SP_UNITS, nc.sync), (ACT_UNITS, nc.scalar),
                         (POOL_UNITS, nc.gpsimd)):
            for u in units:
                lo, hi = u * UNIT, (u + 1) * UNIT
                q.dma_start(out=at_sb[:, lo:hi], in_=at_d[:, lo:hi])

        # Zero the score region up front (DVE is otherwise idle here); the
        # per-graph matmuls then accumulate start=False into disjoint slots
        # with no PSUM group bookkeeping at all.
        nc.vector.memset(sc_ps[:], 0.0)

        # W = wq @ wk^T (f32 inputs), cast to bf16 in SBUF by DVE.
        nc.tensor.matmul(w_ps[:], lhsT=wqwk_sb[:, 0:128],
                         rhs=wqwk_sb[:, 128:256], start=True, stop=True)
        nc.vector.tensor_copy(w_sb[:], w_ps[:])

        # z^T = W^T S^T : [128 d, 2048 g] over four PSUM banks, each staged
        # to SBUF bf16 by DVE as soon as its matmul retires.
        for k in range(4):
            s = slice(512 * k, 512 * (k + 1))
            nc.tensor.matmul(zt_ps[:, s], lhsT=w_sb[:], rhs=st_sb[:, s],
                             start=True, stop=True)
            nc.vector.tensor_copy(z_sb[:, s], zt_ps[:, s])

        # One matmul per graph, accumulating into disjoint zeroed PSUM slots.
        for (n0, w, col, row, g) in _PIECES:
            nc.tensor.matmul(sc_ps[row:row + w, col:col + 1],
                             lhsT=at_sb[:, n0:n0 + w],
                             rhs=z_sb[:, g:g + 1],
                             start=False, stop=False, skip_group_check=True)

        nc.vector.tensor_copy(o_sb[:], sc_ps[:, 0:NCOL])
        # Parallel stores, one per DMA queue.
        c1, c2 = 228, 456
        nc.sync.dma_start(out=out_d[:, 0:c1], in_=o_sb[:, 0:c1])
        nc.scalar.dma_start(out=out_d[:, c1:c2], in_=o_sb[:, c1:c2])
        nc.gpsimd.dma_start(out=out_d[:, c2:NCOL], in_=o_sb[:, c2:NCOL])

    if split_waits:
        _split_multi_waits(nc)
    return nc


def _split_multi_waits(nc):
    """Walrus in this toolchain accepts at most one sync wait on a regular
    instruction (and two on an EventSemaphore). Tile's sem assignment can
    attach several, so strip the excess onto same-engine EventSemaphore
    instructions placed immediately before the owner - same-engine program
    order makes that equivalent."""
    from concourse import mybir
    for fn in nc.m.functions:
        for bb in fn.blocks:
            new = []
            for inst in bb.instructions:
                si = inst.sync_info
                if (si is not None and len(si.on_wait) > 1
                        and not isinstance(inst, mybir.InstEventSemaphore)):
                    waits = list(si.on_wait)
                    keep, rest = waits[-1:], waits[:-1]
                    k = 0
                    while rest:
                        chunk, rest = rest[:2], rest[2:]
                        new.append(mybir.InstEventSemaphore(
                            name=f"{inst.name}-w{k}",
                            engine=inst.engine,
                            sync_info=mybir.SyncInfo(on_wait=chunk,
                                                     on_update=[])))
                        k += 1
                    inst.sync_info = mybir.SyncInfo(
                        on_wait=keep, on_update=list(si.on_update))
                new.append(inst)
            bb.instructions[:] = new


def _get_program():
    global _PROGRAM
    if _PROGRAM is None:
        _PROGRAM = _build_program()
    return _PROGRAM


def _structured(gather_idx, valid_mask, rev_idx):
    """True iff the index tensors match the deterministic ragged layout."""
    counts = COUNTS
    off = np.concatenate([[0], np.cumsum(counts)[:-1]])
    slots = np.arange(M)[None, :]
    valid = (slots < counts[:, None])
    gidx = off[:, None] + np.minimum(slots, counts[:, None] - 1)
    within = np.arange(TOTAL) - np.repeat(off, counts)
    rev = np.repeat(np.arange(B), counts) * M + within
    return (np.array_equal(np.asarray(gather_idx), gidx)
            and np.array_equal(np.asarray(valid_mask), valid.astype(np.float32))
            and np.array_equal(np.asarray(rev_idx), rev))


def _reference_fallback(state_embed, action_embed, wq, wk, gather_idx,
                        valid_mask, rev_idx):
    padded = action_embed[gather_idx] * valid_mask[..., None]
    q = state_embed @ wq
    k = padded @ wk
    scores = np.einsum("bd,bmd->bm", q, k)
    return scores.reshape(-1)[rev_idx][:, None].astype(np.float32)


def _make_in_maps(ins):
    import ml_dtypes
    bf16 = ml_dtypes.bfloat16
    state_embed = np.asarray(ins["state_embed"], np.float32)
    action_embed = np.asarray(ins["action_embed"], np.float32)
    wqwk = np.ascontiguousarray(np.concatenate(
        [np.asarray(ins["wq"], np.float32).T,
         np.asarray(ins["wk"], np.float32).T], axis=1))       # [128, 256] f32
    in_maps = []
    for c in range(NCORES):
        st_c = np.ascontiguousarray(
            state_embed[GPC * c:GPC * (c + 1)].T.astype(bf16))  # [128, 2048]
        at_c = np.ascontiguousarray(
            action_embed[NPC * c:NPC * (c + 1)].T.astype(bf16))  # [128, 25600]
        in_maps.append({"at": at_c, "st": st_c, "wqwk": wqwk})
    return in_maps


def kernel(state_embed, action_embed, wq, wk, gather_idx, valid_mask, rev_idx):
    if not _structured(gather_idx, valid_mask, rev_idx):
        # Inputs deviate from the deterministic ragged layout this kernel is
        # specialized for; fall back to a host computation to stay correct.
        return _reference_fallback(
            np.asarray(state_embed, np.float32),
            np.asarray(action_embed, np.float32),
            np.asarray(wq, np.float32), np.asarray(wk, np.float32),
            np.asarray(gather_idx), np.asarray(valid_mask),
            np.asarray(rev_idx))

    from concourse.bass_utils import run_bass_kernel_spmd

    nc = _get_program()
    in_maps = _make_in_maps({
        "state_embed": state_embed, "action_embed": action_embed,
        "wq": wq, "wk": wk,
    })
    results = run_bass_kernel_spmd(nc, in_maps, list(range(NCORES))).results
    # Gather the 3-band [128, NCOL] layout back to local node order per core.
    out = np.concatenate(
        [np.asarray(results[c]["out"])[_OROWS, _OCOLS] for c in range(NCORES)])
    return out[:, None]
